# revision 50
# baseline (speedup 1.0000x reference)
"""Locally-connected 2D conv (unshared weights), VALID, stride 2 — Trainium2 Bass kernel.

Problem (hardcoded):
  x:       (16, 32, 113, 113) f32
  weights: (56, 56, 32, 3, 3, 64) f32   (H_out, W_out, C_in, kh, kw, C_out)
  bias:    (56, 56, 64) f32
  out:     (16, 64, 56, 56) f32
  out[b,o,u,v] = sum_{c,q,r} x[b,c,2u+q,2v+r] * weights[u,v,c,q,r,o] + bias[u,v,o]

Sharding: H_out split across 8 cores (7 output rows each); each core reads only
its 1/8 of the weight tensor (the dominant traffic).

Design (v13 — int8 weight stream, bias via matmul, pipeline-paced x):
  - DMA-fabric bound (~230 GB/s/core effective with all 8 cores streaming).
    Weights ship as int8 with one scale s per core; s is folded into the x
    pack on the host, so exact int8 values feed a bf16 matmul after an
    on-chip dequant. rel-err ~1.3e-2 vs the 2e-2 gate.
  - bias enters PSUM through one extra K=14 matmul per chunk that STARTS the
    accumulation group: lhsT = bias lanes [14, 128] (partition k carries the
    vp=k bias row of this chunk, free = (vs,o)), rhs = static one-hot
    xid[k, (vp,vs,b)] = s*[vp==k]. PSUM then holds bias + conv, so
    evictions are pure psum->SBUF copies: vs=0 on DVE, vs=1 on ACT.
  - Weight chunks [96, 5376] int8 stream on the sync HWDGE queue (weights
    ONLY — a [1, N] DMA there would be priced at per-partition bytes by the
    tile scheduler's cost model and stall the planned cadence). Dequant
    int8->bf16 splits per chunk: DVE vp 0..8, ACT vp 9..13, in parallel,
    both under the ~2.4us/chunk DMA arrival rate.
  - x pack [96, 7*16*114] bf16 rides the gpsimd/SWDGE queue in per-u
    t-slices (t-slicing is free: the q-row duplication lives across
    partitions, not t): 3 slices load upfront, the rest are emitted behind
    each per-u y-write so the pipeline itself paces them — this smooths the
    weight-chunk arrival rate (an early x burst starves the dequant engines,
    and their lost time becomes an unrecoverable tail backlog).
  - v-pair matmuls: lhsT = [96, 128] (vsel, o); rhs = [96, (vsel, b)];
    1 + 42 matmuls per (u,ch) chunk into one PSUM group [128, 448] f32 (one
    bank). The final chunk is split into a 10-vp + 4-vp pair of psum groups
    (own bias-mm/evictions, DMA cut at the boundary) so only the small
    group's ~2us chain runs after the last weight byte lands.
  - Output staged bf16 in [128, 7*448] (partition = vs*64+o), written back
    per-u on the gpsimd queue (last piece on scalar). Host unpacks to NCHW.
"""

import numpy as np
import ml_dtypes

BF16 = ml_dtypes.bfloat16

B = 16
C_IN = 32
C_OUT = 64
H_OUT = 56
W_OUT = 56
KK = 3
STRIDE = 2
H_IN = 113
W_PAD = 114           # padded input row width (one zero col)
W2 = W_PAD // 2       # 57

N_CORES = 8
U_PER = H_OUT // N_CORES          # 7 output rows per core
ROWS_IN = (U_PER - 1) * STRIDE + KK  # 15 input rows per core
KPART = C_IN * KK                 # 96 contraction partitions (q,c)
VP = 14                           # v-pairs per PSUM chunk
NCH = 2                           # chunks per u  (2*14*2 = 56 = W_OUT)
NCHUNK = U_PER * NCH              # 14 weight chunks
FPV = KK * 2 * C_OUT              # weight elems per vp (384)
WFREE_CH = VP * FPV               # weight free per (u,ch) chunk (5376)
MOUT = 2 * C_OUT                  # psum partitions / lhsT free (128)
NCOL = VP * 2 * B                 # psum columns per chunk (448)
U_LO = 4
ROW_ELEMS = B * W_PAD             # 1824
YROW = NCH * B * VP               # 448 y elems per partition per u

# dequant split: DVE does vp 0..VSPL-1, ACT vp VSPL..13, in parallel
VSPL = 9
NBF = 0               # trailing chunks shipped as bf16 (skip dequant)

_CACHE = {}


def _build():
    import concourse.mybir as mybir
    from concourse import bacc
    from concourse.tile import TileContext

    f32 = mybir.dt.float32
    bf16 = mybir.dt.bfloat16
    i8 = mybir.dt.int8
    nc = bacc.Bacc("TRN2", target_bir_lowering=False, debug=False,
                   num_devices=N_CORES)
    # Host-prepacked tensors (see _pack_core):
    #   xp[p, (t*16+b)*114 + w] = x[b, c, 2*(u0+t)+q, w] * s,  p = q*32+c
    #   wp[u, ch, p, ((vp*3+r)*2+vs)*64+o] = int8(weights[...]/s)
    #   bp[k, ci*128 + vs*64 + o] = bf16(bias[u(ci), ch(ci)*28+k*2+vs, o]/s)
    #   xid[k, (vp, vs, b)] = s * [vp == k]
    #   y[vs*64+o, (u, ch, b, vp)] bf16
    xp_in = nc.dram_tensor("xp", [KPART, U_PER * ROW_ELEMS], bf16,
                           kind="ExternalInput").ap()
    wp_in = nc.dram_tensor("wp", [U_PER, NCH, KPART, WFREE_CH], i8,
                           kind="ExternalInput").ap()
    # last NBF chunks ship as bf16 (no dequant -> no tail drain chain)
    wpb_in = None
    if NBF:
        wpb_in = nc.dram_tensor("wpb", [NBF, KPART, WFREE_CH], bf16,
                                kind="ExternalInput").ap()
    bp_in = nc.dram_tensor("bp", [VP, NCHUNK * MOUT], bf16,
                           kind="ExternalInput").ap()
    xid_in = nc.dram_tensor("xid", [VP, NCOL], bf16,
                            kind="ExternalInput").ap()
    y_out = nc.dram_tensor("y", [2 * C_OUT, U_PER * YROW], bf16,
                           kind="ExternalOutput").ap()

    with TileContext(nc) as tc:
        with tc.tile_pool(name="xpool", bufs=1) as xpool, \
             tc.tile_pool(name="w8pool", bufs=12) as w8pool, \
             tc.tile_pool(name="wpool", bufs=6) as wpool, \
             tc.tile_pool(name="opool", bufs=1) as opool, \
             tc.tile_pool(name="pspool", bufs=8, space="PSUM") as pspool:

            # queue roles: sync = weights only; gpsimd/SWDGE = bias, xid, x
            # slices, y(u<6); scalar = only the final y write. x loads in
            # per-u slices: 3 upfront, the rest emitted behind each y-write
            # so the pipeline itself paces them (smooths chunk arrivals —
            # the early x burst otherwise starves the dequant engines, whose
            # lost time becomes an unrecoverable tail backlog)
            bt = xpool.tile([VP, NCHUNK * MOUT], bf16)
            xit = xpool.tile([VP, NCOL], bf16)
            xf = xpool.tile([KPART, U_PER * ROW_ELEMS], bf16)
            nc.gpsimd.dma_start(out=bt[:], in_=bp_in[:])
            nc.gpsimd.dma_start(out=xit[:], in_=xid_in[:])

            def x_slice(t):
                nc.gpsimd.dma_start(
                    out=xf[:, t * ROW_ELEMS:(t + 1) * ROW_ELEMS],
                    in_=xp_in[:, t * ROW_ELEMS:(t + 1) * ROW_ELEMS])
            for t in range(3):
                x_slice(t)

            oa = opool.tile([2 * C_OUT, U_PER * YROW], bf16)

            # x view: [p, pr, t, w2, b]; w = w2*2 + pr, t = local output row
            xvf = xf.rearrange("p (t b w2 pr) -> p pr t w2 b",
                               t=U_PER, b=B, w2=W2, pr=2)
            # output view: [p(vs,o), u, ch, b, vp]
            ov = oa.rearrange("p (u ch b vp) -> p u ch b vp",
                              u=U_PER, ch=NCH, b=B, vp=VP)

            LAG = 2
            pending = []          # (u, ch, psv, vp0, vp1) awaiting eviction

            def emit_evict(ent):
                # pure psum->SBUF copies (bias already in psum):
                # vs=0 on DVE, vs=1 on ACT
                u, ch, psv, vp0, vp1 = ent
                nc.vector.tensor_copy(
                    out=ov[0:C_OUT, u, ch, :, vp0:vp1],
                    in_=psv[0:C_OUT, :, 0, :])
                nc.scalar.copy(
                    out=ov[C_OUT:2 * C_OUT, u, ch, :, vp0:vp1],
                    in_=psv[C_OUT:2 * C_OUT, :, 1, :])
                if ch == NCH - 1 and vp1 == VP:
                    eng = nc.scalar if u == U_PER - 1 else nc.gpsimd
                    eng.dma_start(
                        out=y_out[:, u * YROW:(u + 1) * YROW],
                        in_=oa[:, u * YROW:(u + 1) * YROW])
                    if u + 3 <= U_PER - 1:
                        x_slice(u + 3)

            for u in range(U_PER):
                xv, xtl = xvf, u
                for ch in range(NCH):
                    ci = u * NCH + ch
                    last = ci == NCHUNK - 1
                    # final chunk: split into a 10-vp and a 4-vp psum group
                    # (own bias-mm + evictions) with the DMA cut at the
                    # boundary, so only the small group's ~2us chain runs
                    # after the last weight byte lands
                    if last:
                        dma_groups = [(0, 10), (10, VP)]
                        dq_splits = [('v', 0, 7), ('a', 7, 10),
                                     ('v', 10, 12), ('a', 12, VP)]
                        mm_groups = [(0, 10), (10, VP)]
                    else:
                        dma_groups = [(0, VP)]
                        dq_splits = [('v', 0, VSPL), ('a', VSPL, VP)]
                        mm_groups = [(0, VP)]
                    wt = wpool.tile([KPART, WFREE_CH], bf16)
                    w8 = w8pool.tile([KPART, WFREE_CH], i8)
                    for dg0, dg1 in dma_groups:
                        nc.sync.dma_start(
                            out=w8[:, dg0 * FPV:dg1 * FPV],
                            in_=wp_in[u, ch, :, dg0 * FPV:dg1 * FPV])
                    for eng, q0, q1 in dq_splits:
                        if eng == 'v':
                            nc.vector.tensor_copy(
                                out=wt[:, q0 * FPV:q1 * FPV],
                                in_=w8[:, q0 * FPV:q1 * FPV])
                        else:
                            nc.scalar.copy(
                                out=wt[:, q0 * FPV:q1 * FPV],
                                in_=w8[:, q0 * FPV:q1 * FPV])
                    # weight view: [p, vp, r, (vs o)]
                    wv = wt.rearrange("p (vp r vs o) -> p vp r (vs o)",
                                      vp=VP, r=KK, vs=2, o=C_OUT)
                    for g0, g1 in mm_groups:
                        nvp = g1 - g0
                        ps = pspool.tile([2 * C_OUT, nvp * 2 * B], f32)
                        # bias matmul starts the group: writes bias[vp,vs,o]
                        # into every (vp, vs, b) psum column
                        nc.tensor.matmul(
                            ps[:], bt[:, ci * MOUT:(ci + 1) * MOUT],
                            xit[:, g0 * 2 * B:g1 * 2 * B],
                            start=True, stop=False)
                        for vp in range(g0, g1):
                            for r in range(KK):
                                v = ch * 2 * VP + vp * 2
                                w2 = v + r // 2
                                rhs = xv[:, r % 2, xtl, w2:w2 + 2, :]
                                nc.tensor.matmul(
                                    ps[:, (vp - g0) * 2 * B:
                                       (vp - g0 + 1) * 2 * B],
                                    wv[:, vp, r], rhs,
                                    start=False,
                                    stop=(vp == g1 - 1 and r == KK - 1),
                                )
                        # psum view: [vs*64+o, b, vs', vp]; diagonal blocks
                        psv = ps.rearrange("p (vp vs b) -> p b vs vp",
                                           vp=nvp, vs=2, b=B)
                        pending.append((u, ch, psv, g0, g1))
                        if len(pending) > LAG:
                            emit_evict(pending.pop(0))
            for ent in pending:
                emit_evict(ent)

    nc.compile()
    return nc


def _get_nc():
    if "nc" not in _CACHE:
        _CACHE["nc"] = _build()
    return _CACHE["nc"]


def _pack_core(x, weights, bias, i):
    u0 = i * U_PER
    s = np.abs(weights[u0:u0 + U_PER]).max() / 127.0

    # x': (96, 7*16*114); p = q*32+c holds rows 2*(u0+t)+q; free (t, b, w);
    # pre-scaled by s
    xs = x[:, :, STRIDE * u0:STRIDE * u0 + ROWS_IN, :]      # (B, C, 15, 113)
    xq = np.stack([xs[:, :, q:q + 2 * U_PER - 1:2, :] for q in range(KK)],
                  axis=0)                                   # (q, B, C, 7, 113)
    xq = xq.transpose(0, 2, 3, 1, 4)                        # (q, c, t, b, w)
    xp = np.zeros((KPART, U_PER, B, W_PAD), dtype=BF16)
    xp[:, :, :, :H_IN] = (xq.reshape(KPART, U_PER, B, H_IN) * s).astype(BF16)
    xp = xp.reshape(KPART, U_PER * ROW_ELEMS)

    # w': (7, 2, 96, 5376) int8; p = q*32+c, free (vp, r, vs, o);
    # the trailing NBF chunks additionally ship as bf16 (x is pre-scaled by
    # s for the int8 chunks, so the bf16 copies store weights/s)
    ws = weights[u0:u0 + U_PER].reshape(U_PER, NCH, VP, 2, C_IN, KK, KK,
                                        C_OUT)                # u ch vp vs c q r o
    ws = ws.transpose(0, 1, 5, 4, 2, 6, 3, 7)                 # u ch q c vp r vs o
    wq = np.clip(np.round(ws / s), -127, 127).astype(np.int8)
    wp = np.ascontiguousarray(wq.reshape(U_PER, NCH, KPART, WFREE_CH))
    wpb = np.ascontiguousarray(
        (ws.reshape(NCHUNK, KPART, WFREE_CH)[NCHUNK - NBF:] / s
         ).astype(BF16)) if NBF else None

    # bias lanes: bp[k, ci*128 + vs*64 + o] = bias[u(ci), ch*28 + k*2 + vs]/s
    bs = bias[u0:u0 + U_PER].reshape(U_PER, NCH, VP, 2, C_OUT)  # u ch vp vs o
    bp = (bs.transpose(2, 0, 1, 3, 4) / s).astype(BF16)         # vp u ch vs o
    bp = np.ascontiguousarray(bp.reshape(VP, NCHUNK * MOUT))

    # one-hot x: xid[k, (vp, vs, b)] = s * [vp == k]
    xid = np.zeros((VP, VP, 2 * B), dtype=BF16)
    for k in range(VP):
        xid[k, k, :] = np.float32(s).astype(BF16)
    xid = xid.reshape(VP, NCOL)
    m = {"xp": xp, "wp": wp, "bp": bp, "xid": xid}
    if NBF:
        m["wpb"] = wpb
    return m


def kernel(x, weights, bias, _trace=False, _tmpdir=None):
    from concourse.bass_utils import run_bass_kernel_spmd

    x = np.ascontiguousarray(x, dtype=np.float32)
    weights = np.ascontiguousarray(weights, dtype=np.float32)
    bias = np.ascontiguousarray(bias, dtype=np.float32)

    nc = _get_nc()
    core_ids = list(range(N_CORES))
    in_maps = [_pack_core(x, weights, bias, i) for i in core_ids]
    res = run_bass_kernel_spmd(nc, in_maps, core_ids, trace=_trace,
                               tmpdir=_tmpdir)
    parts = []
    for i in core_ids:
        y = np.asarray(res.results[i]["y"]).astype(np.float32)
        # y: [vs*64+o, (u, ch, b, vp)] -> (b, o, u, v) with v = ch*28+vp*2+vs
        y = y.reshape(2, C_OUT, U_PER, NCH, B, VP)            # vs o u ch b vp
        y = y.transpose(4, 1, 2, 3, 5, 0)                     # b o u ch vp vs
        parts.append(y.reshape(B, C_OUT, U_PER, W_OUT))
    out = np.concatenate(parts, axis=2)
    if _trace:
        _CACHE["last_result"] = res
    return out
